# revision 1
# baseline (speedup 1.0000x reference)
"""Diagonalizable linear plant (modal state-space scan) on 8 Trainium2 cores.

y[b,t] = Cz @ z[b,t-1] + D @ u[b,t],  z[b,t] = lam * z[b,t-1] + Bz @ u[b,t]
with z[b,-1] = z0[b] = x0[b] @ Q, Bz = Q^T Bmat, Cz = C Q.

Sharding: data-parallel over batch (16 batches -> 2 per core).

Block-8 formulation (the DVE scan instruction runs at ~2 cycles/element,
so the time axis is decimated 8x before it reaches the scan; everything
else is full 128x128xN=512 bf16 matmuls, fp32 PSUM):
  host packs u as uT8[(i*32+u), k] = u[8k+i, u]        (256 rows = 2 K-groups)
  PE   V_h = W2^T @ U          W2[(i,u),n] = lam_n^(7-i) Bz[n,u]
  DVE  zB = scan(lam^8, V)     block-boundary states z_{8k+7}
  PE   Y_g = WC^T @ zBprev + WU^T @ U     (g indexes (j,y) output groups)
       WC[n,(j,y)] = lam_n^j Cz[y,n]
       WU[(i,u),(j,y)] = (Cz lam^(j-1-i) Bz)[y,u] for i<j, D[y,u] for i=j, else 0
  host unpacks yT8[(32j+y), k] -> y[8k+j, y]
"""

import numpy as np

B, T, NX, NU, NY = 16, 8192, 256, 32, 32
NCORES = 8
BPC = B // NCORES   # batches per core
MB = 8              # time-block folded into matmul K
KCOL = T // MB      # block columns per batch (1024)
L = 512             # block-columns per chunk
NCHUNK = KCOL // L  # chunks per batch (2)

_PROG = None  # built Bass program, cached across kernel() calls


def _patch_tile_drain():
    """walrus codegen in this container rejects >1 sync wait on one SP
    TPB_CTRL instruction (terminal TileContext drain / NoOp). Split the
    drain's waits across preceding SP nops carrying one wait each."""
    import concourse.tile as tile
    import concourse.mybir as mybir
    from concourse.vector_clock import ScopedClock

    if getattr(tile.TileContext, "_drain_patched", False):
        return

    def _drain_and_barrier(self, tick_clock, wait_clock):
        nc = self.nc
        scratch = nc.sync.nop()
        wait_clock.add_sem_waits(
            scratch.ins, ScopedClock({None: tick_clock.global_clock})
        )
        si = scratch.ins.sync_info
        waits = list(si.on_wait) if si is not None else []
        scratch.ins.sync_info = mybir.SyncInfo(on_wait=waits[:1], on_update=[])
        for w in waits[1:]:
            n2 = nc.sync.nop()
            n2.ins.sync_info = mybir.SyncInfo(on_wait=[w], on_update=[])
        nc.sync.drain()
        nc.all_engine_barrier()
        assert self.sems is not None
        popped = nc._tile_sem_poison_stack.pop()
        assert popped is self._sem_poison
        nc.clear_and_free_semaphores(list(self.sems.allocated().values()))
        nc.all_engine_barrier()

    tile.TileContext._drain_and_barrier = _drain_and_barrier
    tile.TileContext._drain_patched = True


def _split_multi_waits(nc, mybir):
    """This container's walrus codegen accepts at most ONE sync wait per
    instruction. Hoist extra waits into standalone EventSemaphore nops on
    the same engine, placed immediately before the instruction."""
    ctr = [0]

    def fresh(engine, wait):
        ctr[0] += 1
        ev = mybir.InstEventSemaphore(name=f"I-wsplit-{ctr[0]}", ins=[], outs=[])
        ev.engine = engine
        ev.sync_info = mybir.SyncInfo(on_wait=[wait], on_update=[])
        nc.register_instruction(ev)
        return ev

    for fn in nc.m.functions:
        for bb in fn.blocks:
            out = []
            changed = False
            for inst in bb.instructions:
                si = inst.sync_info
                waits = list(si.on_wait) if si is not None else []
                if len(waits) > 1:
                    changed = True
                    for w in waits[:-1]:
                        out.append(fresh(inst.engine, w))
                    inst.sync_info = mybir.SyncInfo(
                        on_wait=[waits[-1]], on_update=list(si.on_update)
                    )
                out.append(inst)
            if changed:
                bb.instructions = out


def build_program():
    import concourse.bass as bass
    import concourse.tile as tile
    import concourse.mybir as mybir
    from contextlib import ExitStack

    _patch_tile_drain()
    f32 = mybir.dt.float32
    bf = mybir.dt.bfloat16

    nc = bass.Bass()
    uT8 = nc.declare_dram_parameter("uT8", [BPC, 256, KCOL], bf, isOutput=False)
    wAll = nc.declare_dram_parameter("wAll", [128, 12 * 128], bf, isOutput=False)
    z0c = nc.declare_dram_parameter("z0c", [128, 2 * BPC], f32, isOutput=False)
    lam8c = nc.declare_dram_parameter("lam8c", [128, 2], f32, isOutput=False)
    yT8 = nc.declare_dram_parameter("yT8", [BPC, 256, KCOL], bf, isOutput=True)

    with ExitStack() as ctx:
        tc = ctx.enter_context(tile.TileContext(nc))
        const = ctx.enter_context(tc.tile_pool(name="const", bufs=1))
        upool = ctx.enter_context(tc.tile_pool(name="u", bufs=3))
        vps = ctx.enter_context(tc.tile_pool(name="vps", bufs=2, space="PSUM"))
        yps = ctx.enter_context(tc.tile_pool(name="yps", bufs=2, space="PSUM"))
        zpool = ctx.enter_context(tc.tile_pool(name="z", bufs=6))
        yout = ctx.enter_context(tc.tile_pool(name="yo", bufs=4))

        # tiny params first (lam gates the first scan), then v-matmul
        # weights, then the rest — all on the SP HWDGE queue; first chunk's
        # U tiles in parallel on the other two queues
        lam8t = const.tile([128, 2], f32)
        nc.sync.dma_start(lam8t[:], lam8c[:])
        z0t = const.tile([128, 2 * BPC], f32)
        nc.sync.dma_start(z0t[:], z0c[:])
        W2t = const.tile([128, 512], bf)
        nc.sync.dma_start(W2t[:], wAll[:, 0:512])
        U0_first = upool.tile([128, L], bf, name="U0f", tag="U0")
        nc.scalar.dma_start(U0_first[:], uT8[0, 0:128, 0:L])
        U1_first = upool.tile([128, L], bf, name="U1f", tag="U1")
        nc.gpsimd.dma_start(U1_first[:], uT8[0, 128:256, 0:L])
        WCUt = const.tile([128, 1024], bf)
        nc.sync.dma_start(WCUt[:], wAll[:, 512:1536])

        # PE warm-up: dependency-free matmuls trip the HAM clock gate to
        # 2.4 GHz right as the first real operands land (borrows a V slot)
        dummy = const.tile([128, L], bf)
        nc.vector.memset(dummy[:], 0.0)
        WP = vps.tile([128, L], f32, name="WP", tag="V0")
        for _ in range(5):
            nc.tensor.matmul(WP[:], lhsT=dummy[:, 0:128], rhs=dummy[:],
                             start=True, stop=True)

        # lam broadcast built on DVE during the DMA warm-up window
        ones = const.tile([128, L], f32)
        nc.vector.memset(ones[:], 1.0)
        lam_bc = const.tile([128, 2 * L], f32)
        for h in range(2):
            nc.vector.tensor_scalar_mul(
                lam_bc[:, h * L : (h + 1) * L], ones[:], lam8t[:, h : h + 1]
            )

        def w2blk(i):
            return W2t[:, 128 * i : 128 * (i + 1)]

        def wcublk(i):
            return WCUt[:, 128 * i : 128 * (i + 1)]

        W2 = [[w2blk(0), w2blk(1)], [w2blk(2), w2blk(3)]]      # [g][h]
        WC = [[wcublk(0), wcublk(1)], [wcublk(2), wcublk(3)]]  # [h][g]
        WU = [[wcublk(4), wcublk(5)], [wcublk(6), wcublk(7)]]  # [g2][g]

        mult = mybir.AluOpType.mult
        add = mybir.AluOpType.add

        prev_z = [[None, None] for _ in range(BPC)]

        def emit_vscan(c, b):
            sl = slice(c * L, (c + 1) * L)
            if b == 0 and c == 0:
                U = [U0_first, U1_first]
            else:
                U = []
                for g in range(2):
                    t = upool.tile([128, L], bf, name=f"U{g}_{b}_{c}",
                                   tag=f"U{g}")
                    eng = nc.scalar if g == 0 else nc.gpsimd
                    eng.dma_start(t[:], uT8[b, 128 * g : 128 * (g + 1), sl])
                    U.append(t)
            zext = [None, None]
            for h in range(2):
                V = vps.tile([128, L], f32, name=f"V{h}_{b}_{c}", tag=f"V{h}")
                nc.tensor.matmul(V[:], lhsT=W2[0][h], rhs=U[0][:],
                                 start=True, stop=False)
                nc.tensor.matmul(V[:], lhsT=W2[1][h], rhs=U[1][:],
                                 start=False, stop=True)
                Z = zpool.tile([128, L + 1], bf, name=f"Z{h}_{b}_{c}",
                               tag=f"Z{h}")
                if c == 0:
                    carry = z0t[:, 2 * b + h : 2 * b + h + 1]
                else:
                    carry = prev_z[b][h][:, L : L + 1]
                nc.vector.tensor_tensor_scan(
                    Z[:, 1 : L + 1], lam_bc[:, h * L : (h + 1) * L], V[:],
                    carry, mult, add,
                )
                nc.scalar.copy(Z[:, 0:1], carry)
                zext[h] = Z
            prev_z[b] = zext
            return U, zext

        def emit_y(c, b, U, zext):
            sl = slice(c * L, (c + 1) * L)
            for g in range(2):
                Y = yps.tile([128, L], f32, name=f"Y{g}_{b}_{c}", tag=f"Y{g}")
                nc.tensor.matmul(Y[:], lhsT=WU[0][g], rhs=U[0][:],
                                 start=True, stop=False)
                nc.tensor.matmul(Y[:], lhsT=WU[1][g], rhs=U[1][:],
                                 start=False, stop=False)
                nc.tensor.matmul(Y[:], lhsT=WC[0][g], rhs=zext[0][:, 0:L],
                                 start=False, stop=False)
                nc.tensor.matmul(Y[:], lhsT=WC[1][g], rhs=zext[1][:, 0:L],
                                 start=False, stop=True)
                Ysb = yout.tile([128, L], bf, name=f"Ysb{g}_{b}_{c}",
                                tag=f"Ysb{g}")
                nc.scalar.copy(Ysb[:], Y[:])
                oeng = nc.sync if g == 0 else nc.gpsimd
                oeng.dma_start(yT8[b, 128 * g : 128 * (g + 1), sl], Ysb[:])

        units = [(c, b) for c in range(NCHUNK) for b in range(BPC)]
        pending = []
        for (c, b) in units:
            U, zext = emit_vscan(c, b)
            pending.append((c, b, U, zext))
            if len(pending) > 2:
                emit_y(*pending.pop(0))
        for p in pending:
            emit_y(*p)

    _split_multi_waits(nc, mybir)
    return nc


def _host_prep(x0, u, Q, lam, Bmat, C, D):
    import ml_dtypes

    f = np.float32
    bfd = ml_dtypes.bfloat16
    lam = lam.astype(f)
    Bz = (Q.T.astype(f) @ Bmat.astype(f)).astype(f)      # (NX, NU)
    Cz = (C.astype(f) @ Q.astype(f)).astype(f)           # (NY, NX)
    z0 = (x0.astype(f) @ Q.astype(f)).astype(f)          # (B, NX)

    lam_p = np.stack([lam**j for j in range(MB)])         # (MB, NX)

    # W2[(i*32+u), n] = lam_n^(MB-1-i) * Bz[n, u]
    W2 = np.einsum("in,nu->iun", lam_p[::-1], Bz).reshape(MB * NU, NX)
    # WC[n, (32j+y)] = lam_n^j * Cz[y, n]
    WC = np.einsum("jn,yn->njy", lam_p, Cz).reshape(NX, MB * NY)
    # WU[(i*32+u), (32j+y)]
    WU = np.zeros((MB * NU, MB * NY), dtype=f)
    for j in range(MB):
        for i in range(MB):
            if i < j:
                Mji = (Cz * lam_p[j - 1 - i][None, :]) @ Bz   # (NY, NU)
                WU[i * NU : (i + 1) * NU, j * NY : (j + 1) * NY] = Mji.T
            elif i == j:
                WU[i * NU : (i + 1) * NU, j * NY : (j + 1) * NY] = D.T.astype(f)

    blocks = []
    for g in range(2):          # W2[g][h]
        for h in range(2):
            blocks.append(W2[128 * g : 128 * (g + 1), 128 * h : 128 * (h + 1)])
    for h in range(2):          # WC[h][g]
        for g in range(2):
            blocks.append(WC[128 * h : 128 * (h + 1), 128 * g : 128 * (g + 1)])
    for g2 in range(2):         # WU[g2][g]
        for g in range(2):
            blocks.append(WU[128 * g2 : 128 * (g2 + 1), 128 * g : 128 * (g + 1)])
    wAll = np.concatenate(blocks, axis=1).astype(bfd)     # (128, 12*128)

    # uT8[b][(i*32+u), k] = u[b, 8k+i, u]
    uT8 = np.ascontiguousarray(
        u.reshape(B, KCOL, MB, NU).transpose(0, 2, 3, 1).reshape(B, MB * NU, KCOL)
    ).astype(bfd)

    lam8 = lam**MB
    lam8c = np.ascontiguousarray(np.stack([lam8[:128], lam8[128:]], axis=1)).astype(f)
    return wAll, z0, uT8, lam8c


def make_in_maps(x0, u, Q, lam, Bmat, C, D):
    wAll, z0, uT8, lam8c = _host_prep(x0, u, Q, lam, Bmat, C, D)
    in_maps = []
    for cidx in range(NCORES):
        sl = slice(cidx * BPC, (cidx + 1) * BPC)
        z0_c = z0[sl]
        z0c = np.ascontiguousarray(
            z0_c.reshape(BPC, 2, 128).transpose(2, 0, 1).reshape(128, 2 * BPC)
        )
        in_maps.append(
            {
                "uT8": np.ascontiguousarray(uT8[sl]),
                "wAll": wAll,
                "z0c": z0c,
                "lam8c": lam8c,
            }
        )
    return in_maps


def kernel(x0, u, Q, lam, Bmat, C, D):
    global _PROG
    from concourse.bass_utils import run_bass_kernel_spmd

    if _PROG is None:
        _PROG = build_program()
    in_maps = make_in_maps(x0, u, Q, lam, Bmat, C, D)
    res = run_bass_kernel_spmd(_PROG, in_maps, list(range(NCORES)))
    y = np.empty((B, T, NY), dtype=np.float32)
    for cidx in range(NCORES):
        yT8_c = res.results[cidx]["yT8"].astype(np.float32)  # (BPC, 256, KCOL)
        # y[b, 8k+j, yy] = yT8[b, 32j+yy, k]
        y[cidx * BPC : (cidx + 1) * BPC] = (
            yT8_c.reshape(BPC, MB, NY, KCOL).transpose(0, 3, 1, 2).reshape(BPC, T, NY)
        )
    return y



# revision 4
# speedup vs baseline: 1.2294x; 1.2294x over previous
"""Diagonalizable linear plant (modal state-space scan) on 8 Trainium2 cores.

y[b,t] = Cz @ z[b,t-1] + D @ u[b,t],  z[b,t] = lam * z[b,t-1] + Bz @ u[b,t]
with z[b,-1] = z0[b] = x0[b] @ Q, Bz = Q^T Bmat, Cz = C Q.

Sharding: data-parallel over batch (16 batches -> 2 per core).

Block-8 formulation (the DVE scan instruction runs at ~2 cycles/element,
so the time axis is decimated 8x before it reaches the scan; everything
else is full 128x128xN=512 bf16 matmuls, fp32 PSUM):
  host packs u as uT8[(i*32+u), k] = u[8k+i, u]        (256 rows = 2 K-groups)
  PE   V_h = W2^T @ U          W2[(i,u),n] = lam_n^(7-i) Bz[n,u]
  DVE  zB = scan(lam^8, V)     block-boundary states z_{8k+7}
  PE   Y_g = WC^T @ zBprev + WU^T @ U     (g indexes (j,y) output groups)
       WC[n,(j,y)] = lam_n^j Cz[y,n]
       WU[(i,u),(j,y)] = (Cz lam^(j-1-i) Bz)[y,u] for i<j, D[y,u] for i=j, else 0
  host unpacks yT8[(32j+y), k] -> y[8k+j, y]
"""

import numpy as np

B, T, NX, NU, NY = 16, 8192, 256, 32, 32
NCORES = 8
BPC = B // NCORES   # batches per core
MB = 8              # time-block folded into matmul K
KCOL = T // MB      # block columns per batch (1024)
L = 512             # block-columns per chunk
NCHUNK = KCOL // L  # chunks per batch (2)

_PROG = None  # built Bass program, cached across kernel() calls


def _patch_tile_drain():
    """walrus codegen in this container rejects >1 sync wait on one SP
    TPB_CTRL instruction (terminal TileContext drain / NoOp). Split the
    drain's waits across preceding SP nops carrying one wait each."""
    import concourse.tile as tile
    import concourse.mybir as mybir
    from concourse.vector_clock import ScopedClock

    if getattr(tile.TileContext, "_drain_patched", False):
        return

    def _drain_and_barrier(self, tick_clock, wait_clock):
        nc = self.nc
        scratch = nc.sync.nop()
        wait_clock.add_sem_waits(
            scratch.ins, ScopedClock({None: tick_clock.global_clock})
        )
        si = scratch.ins.sync_info
        waits = list(si.on_wait) if si is not None else []
        scratch.ins.sync_info = mybir.SyncInfo(on_wait=waits[:1], on_update=[])
        for w in waits[1:]:
            n2 = nc.sync.nop()
            n2.ins.sync_info = mybir.SyncInfo(on_wait=[w], on_update=[])
        nc.sync.drain()
        nc.all_engine_barrier()
        assert self.sems is not None
        popped = nc._tile_sem_poison_stack.pop()
        assert popped is self._sem_poison
        nc.clear_and_free_semaphores(list(self.sems.allocated().values()))
        nc.all_engine_barrier()

    tile.TileContext._drain_and_barrier = _drain_and_barrier
    tile.TileContext._drain_patched = True


def _split_multi_waits(nc, mybir):
    """This container's walrus codegen accepts at most ONE sync wait per
    instruction. Hoist extra waits into standalone EventSemaphore nops on
    the same engine, placed immediately before the instruction."""
    ctr = [0]

    def fresh(engine, wait):
        ctr[0] += 1
        ev = mybir.InstEventSemaphore(name=f"I-wsplit-{ctr[0]}", ins=[], outs=[])
        ev.engine = engine
        ev.sync_info = mybir.SyncInfo(on_wait=[wait], on_update=[])
        nc.register_instruction(ev)
        return ev

    for fn in nc.m.functions:
        for bb in fn.blocks:
            out = []
            changed = False
            for inst in bb.instructions:
                si = inst.sync_info
                waits = list(si.on_wait) if si is not None else []
                if len(waits) > 1:
                    changed = True
                    for w in waits[:-1]:
                        out.append(fresh(inst.engine, w))
                    inst.sync_info = mybir.SyncInfo(
                        on_wait=[waits[-1]], on_update=list(si.on_update)
                    )
                out.append(inst)
            if changed:
                bb.instructions = out


def build_program():
    import concourse.bass as bass
    import concourse.tile as tile
    import concourse.mybir as mybir
    from contextlib import ExitStack

    _patch_tile_drain()
    f32 = mybir.dt.float32
    bf = mybir.dt.bfloat16

    nc = bass.Bass()
    uT8 = nc.declare_dram_parameter("uT8", [BPC, 256, KCOL], bf, isOutput=False)
    wAll = nc.declare_dram_parameter("wAll", [128, 11 * 128], bf, isOutput=False)
    z0c = nc.declare_dram_parameter("z0c", [128, 2 * BPC], f32, isOutput=False)
    lam8c = nc.declare_dram_parameter("lam8c", [128, 2], f32, isOutput=False)
    yT8 = nc.declare_dram_parameter("yT8", [BPC, 256, KCOL], bf, isOutput=True)

    with ExitStack() as ctx:
        tc = ctx.enter_context(tile.TileContext(nc))
        const = ctx.enter_context(tc.tile_pool(name="const", bufs=1))
        vps = ctx.enter_context(tc.tile_pool(name="vps", bufs=2, space="PSUM"))
        yps = ctx.enter_context(tc.tile_pool(name="yps", bufs=2, space="PSUM"))
        zpool = ctx.enter_context(tc.tile_pool(name="z", bufs=6))
        yout = ctx.enter_context(tc.tile_pool(name="yo", bufs=4))

        # DMA plan: the two HWDGE queues (scalar=Activation, sync=SP) carry
        # all bulk traffic — whole-batch U tiles up front, Y stores later.
        # The slow software gpsimd queue gets only the Y-phase weights,
        # which stream in the background and aren't needed until ~+5us.
        W2t = const.tile([128, 512], bf)
        nc.scalar.dma_start(W2t[:], wAll[:, 0:512])
        lam8t = const.tile([128, 2], f32)
        nc.sync.dma_start(lam8t[:], lam8c[:])
        z0t = const.tile([128, 2 * BPC], f32)
        nc.sync.dma_start(z0t[:], z0c[:])
        WCUt = const.tile([128, 896], bf)
        nc.gpsimd.dma_start(WCUt[:], wAll[:, 512:1408])
        Ubig = [[None, None] for _ in range(BPC)]   # [b][g] -> [128, KCOL]
        for b in range(BPC):
            for g in range(2):
                t = const.tile([128, KCOL], bf, name=f"U{g}b{b}")
                eng = nc.scalar if g == 0 else nc.sync
                eng.dma_start(t[:], uT8[b, 128 * g : 128 * (g + 1), :])
                Ubig[b][g] = t

        # PE warm-up during the DMA fill window
        dummy = const.tile([128, L], bf)
        nc.vector.memset(dummy[:], 0.0)
        WP = vps.tile([128, L], f32, name="WP", tag="V0")
        for _ in range(2):
            nc.tensor.matmul(WP[:], lhsT=dummy[:, 0:128], rhs=dummy[:],
                             start=True, stop=True)

        # lam broadcast built on DVE during the DMA fill window
        ones = const.tile([128, L], f32)
        nc.vector.memset(ones[:], 1.0)
        lam_bc = const.tile([128, 2 * L], f32)
        for h in range(2):
            nc.vector.tensor_scalar_mul(
                lam_bc[:, h * L : (h + 1) * L], ones[:], lam8t[:, h : h + 1]
            )

        def w2blk(i):
            return W2t[:, 128 * i : 128 * (i + 1)]

        def wcublk(i):
            return WCUt[:, 128 * i : 128 * (i + 1)]

        W2 = [[w2blk(0), w2blk(1)], [w2blk(2), w2blk(3)]]      # [g][h]
        WC = [[wcublk(0), wcublk(1)], [wcublk(2), wcublk(3)]]  # [h][g]
        WU00, WU01, WU11 = wcublk(4), wcublk(5), wcublk(6)     # WU[1][0] == 0

        mult = mybir.AluOpType.mult
        add = mybir.AluOpType.add

        prev_z = [[None, None] for _ in range(BPC)]

        def emit_vscan(c, b):
            sl = slice(c * L, (c + 1) * L)
            U = [Ubig[b][0][:, sl], Ubig[b][1][:, sl]]
            zext = [None, None]
            for h in range(2):
                V = vps.tile([128, L], f32, name=f"V{h}_{b}_{c}", tag=f"V{h}")
                nc.tensor.matmul(V[:], lhsT=W2[0][h], rhs=U[0],
                                 start=True, stop=False)
                nc.tensor.matmul(V[:], lhsT=W2[1][h], rhs=U[1],
                                 start=False, stop=True)
                Z = zpool.tile([128, L + 1], bf, name=f"Z{h}_{b}_{c}",
                               tag=f"Z{h}")
                if c == 0:
                    carry = z0t[:, 2 * b + h : 2 * b + h + 1]
                else:
                    carry = prev_z[b][h][:, L : L + 1]
                nc.vector.tensor_tensor_scan(
                    Z[:, 1 : L + 1], lam_bc[:, h * L : (h + 1) * L], V[:],
                    carry, mult, add,
                )
                nc.scalar.copy(Z[:, 0:1], carry)
                zext[h] = Z
            prev_z[b] = zext
            return U, zext

        def emit_y(c, b, U, zext):
            sl = slice(c * L, (c + 1) * L)
            for g in range(2):
                Y = yps.tile([128, L], f32, name=f"Y{g}_{b}_{c}", tag=f"Y{g}")
                if g == 0:
                    nc.tensor.matmul(Y[:], lhsT=WU00, rhs=U[0],
                                     start=True, stop=False)
                else:
                    nc.tensor.matmul(Y[:], lhsT=WU01, rhs=U[0],
                                     start=True, stop=False)
                    nc.tensor.matmul(Y[:], lhsT=WU11, rhs=U[1],
                                     start=False, stop=False)
                nc.tensor.matmul(Y[:], lhsT=WC[0][g], rhs=zext[0][:, 0:L],
                                 start=False, stop=False)
                nc.tensor.matmul(Y[:], lhsT=WC[1][g], rhs=zext[1][:, 0:L],
                                 start=False, stop=True)
                Ysb = yout.tile([128, L], bf, name=f"Ysb{g}_{b}_{c}",
                                tag=f"Ysb{g}")
                if g == 0:
                    nc.scalar.copy(Ysb[:], Y[:])
                    nc.sync.dma_start(yT8[b, 0:128, sl], Ysb[:])
                else:
                    nc.vector.tensor_copy(Ysb[:], Y[:])
                    nc.scalar.dma_start(yT8[b, 128:256, sl], Ysb[:])

        units = [(c, b) for c in range(NCHUNK) for b in range(BPC)]
        pending = []
        for (c, b) in units:
            U, zext = emit_vscan(c, b)
            pending.append((c, b, U, zext))
            if len(pending) > 2:
                emit_y(*pending.pop(0))
        for p in pending:
            emit_y(*p)

    _split_multi_waits(nc, mybir)
    return nc


def _host_prep(x0, u, Q, lam, Bmat, C, D):
    import ml_dtypes

    f = np.float32
    bfd = ml_dtypes.bfloat16
    lam = lam.astype(f)
    Bz = (Q.T.astype(f) @ Bmat.astype(f)).astype(f)      # (NX, NU)
    Cz = (C.astype(f) @ Q.astype(f)).astype(f)           # (NY, NX)
    z0 = (x0.astype(f) @ Q.astype(f)).astype(f)          # (B, NX)

    lam_p = np.stack([lam**j for j in range(MB)])         # (MB, NX)

    # W2[(i*32+u), n] = lam_n^(MB-1-i) * Bz[n, u]
    W2 = np.einsum("in,nu->iun", lam_p[::-1], Bz).reshape(MB * NU, NX)
    # WC[n, (32j+y)] = lam_n^j * Cz[y, n]
    WC = np.einsum("jn,yn->njy", lam_p, Cz).reshape(NX, MB * NY)
    # WU[(i*32+u), (32j+y)]
    WU = np.zeros((MB * NU, MB * NY), dtype=f)
    for j in range(MB):
        for i in range(MB):
            if i < j:
                Mji = (Cz * lam_p[j - 1 - i][None, :]) @ Bz   # (NY, NU)
                WU[i * NU : (i + 1) * NU, j * NY : (j + 1) * NY] = Mji.T
            elif i == j:
                WU[i * NU : (i + 1) * NU, j * NY : (j + 1) * NY] = D.T.astype(f)

    blocks = []
    for g in range(2):          # W2[g][h]
        for h in range(2):
            blocks.append(W2[128 * g : 128 * (g + 1), 128 * h : 128 * (h + 1)])
    for h in range(2):          # WC[h][g]
        for g in range(2):
            blocks.append(WC[128 * h : 128 * (h + 1), 128 * g : 128 * (g + 1)])
    # WU[g2][g] blocks; WU[1][0] is identically zero (i > j) and skipped
    blocks.append(WU[0:128, 0:128])      # WU00
    blocks.append(WU[0:128, 128:256])    # WU01
    blocks.append(WU[128:256, 128:256])  # WU11
    wAll = np.concatenate(blocks, axis=1).astype(bfd)     # (128, 11*128)

    # uT8[b][(i*32+u), k] = u[b, 8k+i, u]
    uT8 = np.ascontiguousarray(
        u.reshape(B, KCOL, MB, NU).transpose(0, 2, 3, 1).reshape(B, MB * NU, KCOL)
    ).astype(bfd)

    lam8 = lam**MB
    lam8c = np.ascontiguousarray(np.stack([lam8[:128], lam8[128:]], axis=1)).astype(f)
    return wAll, z0, uT8, lam8c


def make_in_maps(x0, u, Q, lam, Bmat, C, D):
    wAll, z0, uT8, lam8c = _host_prep(x0, u, Q, lam, Bmat, C, D)
    in_maps = []
    for cidx in range(NCORES):
        sl = slice(cidx * BPC, (cidx + 1) * BPC)
        z0_c = z0[sl]
        z0c = np.ascontiguousarray(
            z0_c.reshape(BPC, 2, 128).transpose(2, 0, 1).reshape(128, 2 * BPC)
        )
        in_maps.append(
            {
                "uT8": np.ascontiguousarray(uT8[sl]),
                "wAll": wAll,
                "z0c": z0c,
                "lam8c": lam8c,
            }
        )
    return in_maps


def kernel(x0, u, Q, lam, Bmat, C, D):
    global _PROG
    from concourse.bass_utils import run_bass_kernel_spmd

    if _PROG is None:
        _PROG = build_program()
    in_maps = make_in_maps(x0, u, Q, lam, Bmat, C, D)
    res = run_bass_kernel_spmd(_PROG, in_maps, list(range(NCORES)))
    y = np.empty((B, T, NY), dtype=np.float32)
    for cidx in range(NCORES):
        yT8_c = res.results[cidx]["yT8"].astype(np.float32)  # (BPC, 256, KCOL)
        # y[b, 8k+j, yy] = yT8[b, 32j+yy, k]
        y[cidx * BPC : (cidx + 1) * BPC] = (
            yT8_c.reshape(BPC, MB, NY, KCOL).transpose(0, 3, 1, 2).reshape(BPC, T, NY)
        )
    return y



# revision 10
# speedup vs baseline: 1.2502x; 1.0170x over previous
"""Diagonalizable linear plant (modal state-space scan) on 8 Trainium2 cores.

y[b,t] = Cz @ z[b,t-1] + D @ u[b,t],  z[b,t] = lam * z[b,t-1] + Bz @ u[b,t]
with z[b,-1] = z0[b] = x0[b] @ Q, Bz = Q^T Bmat, Cz = C Q.

Sharding: data-parallel over batch (16 batches -> 2 per core).

Block-8 formulation (the DVE scan instruction runs at ~2 cycles/element,
so the time axis is decimated 8x before it reaches the scan; everything
else is full 128x128xN=512 bf16 matmuls, fp32 PSUM):
  host packs u as uT8[(i*32+u), k] = u[8k+i, u]        (256 rows = 2 K-groups)
  PE   V_h = W2^T @ U          W2[(i,u),n] = lam_n^(7-i) Bz[n,u]
  DVE  zB = scan(lam^8, V)     block-boundary states z_{8k+7}
  PE   Y_g = WC^T @ zBprev + WU^T @ U     (g indexes (j,y) output groups)
       WC[n,(j,y)] = lam_n^j Cz[y,n]
       WU[(i,u),(j,y)] = (Cz lam^(j-1-i) Bz)[y,u] for i<j, D[y,u] for i=j, else 0
  host unpacks yT8[(32j+y), k] -> y[8k+j, y]
"""

import numpy as np

B, T, NX, NU, NY = 16, 8192, 256, 32, 32
NCORES = 8
BPC = B // NCORES   # batches per core
MB = 8              # time-block folded into matmul K
KCOL = T // MB      # block columns per batch (1024)
L = 512             # block-columns per chunk
NCHUNK = KCOL // L  # chunks per batch (2)

_PROG = None  # built Bass program, cached across kernel() calls


def _patch_tile_drain():
    """walrus codegen in this container rejects >1 sync wait on one SP
    TPB_CTRL instruction (terminal TileContext drain / NoOp). Split the
    drain's waits across preceding SP nops carrying one wait each."""
    import concourse.tile as tile
    import concourse.mybir as mybir
    from concourse.vector_clock import ScopedClock

    if getattr(tile.TileContext, "_drain_patched", False):
        return

    def _drain_and_barrier(self, tick_clock, wait_clock):
        nc = self.nc
        scratch = nc.sync.nop()
        wait_clock.add_sem_waits(
            scratch.ins, ScopedClock({None: tick_clock.global_clock})
        )
        si = scratch.ins.sync_info
        waits = list(si.on_wait) if si is not None else []
        scratch.ins.sync_info = mybir.SyncInfo(on_wait=waits[:1], on_update=[])
        for w in waits[1:]:
            n2 = nc.sync.nop()
            n2.ins.sync_info = mybir.SyncInfo(on_wait=[w], on_update=[])
        nc.sync.drain()
        nc.all_engine_barrier()
        assert self.sems is not None
        popped = nc._tile_sem_poison_stack.pop()
        assert popped is self._sem_poison
        nc.clear_and_free_semaphores(list(self.sems.allocated().values()))
        nc.all_engine_barrier()

    tile.TileContext._drain_and_barrier = _drain_and_barrier
    tile.TileContext._drain_patched = True


def _split_multi_waits(nc, mybir):
    """This container's walrus codegen accepts at most ONE sync wait per
    instruction. Hoist extra waits into standalone EventSemaphore nops on
    the same engine, placed immediately before the instruction."""
    ctr = [0]

    def fresh(engine, wait):
        ctr[0] += 1
        ev = mybir.InstEventSemaphore(name=f"I-wsplit-{ctr[0]}", ins=[], outs=[])
        ev.engine = engine
        ev.sync_info = mybir.SyncInfo(on_wait=[wait], on_update=[])
        nc.register_instruction(ev)
        return ev

    for fn in nc.m.functions:
        for bb in fn.blocks:
            out = []
            changed = False
            for inst in bb.instructions:
                si = inst.sync_info
                waits = list(si.on_wait) if si is not None else []
                if len(waits) > 1:
                    changed = True
                    for w in waits[:-1]:
                        out.append(fresh(inst.engine, w))
                    inst.sync_info = mybir.SyncInfo(
                        on_wait=[waits[-1]], on_update=list(si.on_update)
                    )
                out.append(inst)
            if changed:
                bb.instructions = out


def build_program():
    import concourse.bass as bass
    import concourse.tile as tile
    import concourse.mybir as mybir
    from contextlib import ExitStack

    _patch_tile_drain()
    f32 = mybir.dt.float32
    bf = mybir.dt.bfloat16

    nc = bass.Bass()
    # uT8s[b, ch, (i*32+u), k] = u-block columns, ch = column-half so each
    # (b, g, ch) DMA reads a fully contiguous 128 KB region (4 KB packets)
    uT8s = nc.declare_dram_parameter("uT8s", [BPC, 2, 256, L], bf, isOutput=False)
    wAll = nc.declare_dram_parameter("wAll", [128, 11 * 128], bf, isOutput=False)
    # pz: col 0:2 = lam^8 halves, cols 2: = z0 modal states (merged tiny DMA)
    pz = nc.declare_dram_parameter("pz", [128, 2 + 2 * BPC], f32, isOutput=False)
    yT8s = nc.declare_dram_parameter("yT8s", [BPC, 2, 256, L], bf, isOutput=True)

    with ExitStack() as ctx:
        tc = ctx.enter_context(tile.TileContext(nc))
        const = ctx.enter_context(tc.tile_pool(name="const", bufs=1))
        vps = ctx.enter_context(tc.tile_pool(name="vps", bufs=2, space="PSUM"))
        yps = ctx.enter_context(tc.tile_pool(name="yps", bufs=2, space="PSUM"))
        zpool = ctx.enter_context(tc.tile_pool(name="z", bufs=6))
        yout = ctx.enter_context(tc.tile_pool(name="yo", bufs=4))

        # DMA plan: the two HWDGE queues (scalar=Activation, sync=SP) carry
        # all bulk traffic, balanced so the first unit's operands (W2 for
        # h0+h1, U halves for b0) land earliest on both queues. The slow
        # software gpsimd queue gets only the Y-phase weights, which
        # stream in the background and aren't needed until ~+6us.
        W2t = const.tile([128, 512], bf)
        pzt = const.tile([128, 2 + 2 * BPC], f32)
        nc.sync.dma_start(pzt[:], pz[:])
        nc.scalar.dma_start(W2t[:, 0:256], wAll[:, 0:256])
        nc.sync.dma_start(W2t[:, 256:512], wAll[:, 256:512])
        WCUt = const.tile([128, 896], bf)
        nc.gpsimd.dma_start(WCUt[:], wAll[:, 512:1408])
        lam8t = pzt[:, 0:2]
        z0t = pzt[:, 2 : 2 + 2 * BPC]
        # U tiles: [b][g] -> [128, KCOL]; loaded as column-halves, b0 first
        Ubig = [[const.tile([128, KCOL], bf, name=f"U{g}b{b}") for g in range(2)]
                for b in range(BPC)]
        for b in range(BPC):
            for ch in range(2):
                sl = slice(ch * L, (ch + 1) * L)
                nc.scalar.dma_start(Ubig[b][0][:, sl], uT8s[b, ch, 0:128, :])
                nc.sync.dma_start(Ubig[b][1][:, sl], uT8s[b, ch, 128:256, :])

        # PE warm-up matmuls fill the whole DMA window back-to-back so the
        # clock governor sees sustained activity before real work begins
        dummy = const.tile([128, L], bf)
        nc.vector.memset(dummy[:], 0.0)
        WP = vps.tile([128, L], f32, name="WP", tag="V0")
        for _ in range(4):
            nc.tensor.matmul(WP[:], lhsT=dummy[:, 0:128], rhs=dummy[:],
                             start=True, stop=True)

        # lam broadcast built on DVE during the DMA fill window
        ones = const.tile([128, L], f32)
        nc.vector.memset(ones[:], 1.0)
        lam_bc = const.tile([128, 2 * L], f32)
        for h in range(2):
            nc.vector.tensor_scalar_mul(
                lam_bc[:, h * L : (h + 1) * L], ones[:], lam8t[:, h : h + 1]
            )

        def w2blk(i):
            return W2t[:, 128 * i : 128 * (i + 1)]

        def wcublk(i):
            return WCUt[:, 128 * i : 128 * (i + 1)]

        # wAll W2 block order: [g0h0, g1h0, g0h1, g1h1] so the h=0 pair
        # rides the scalar queue (first half) and h=1 the sync queue
        W2 = [[w2blk(0), w2blk(2)], [w2blk(1), w2blk(3)]]      # [g][h]
        WC = [[wcublk(0), wcublk(1)], [wcublk(2), wcublk(3)]]  # [h][g]
        WU00, WU01, WU11 = wcublk(4), wcublk(5), wcublk(6)     # WU[1][0] == 0

        mult = mybir.AluOpType.mult
        add = mybir.AluOpType.add

        prev_z = [[None, None] for _ in range(BPC)]

        def emit_vscan(c, b):
            sl = slice(c * L, (c + 1) * L)
            U = [Ubig[b][0][:, sl], Ubig[b][1][:, sl]]
            zext = [None, None]
            for h in range(2):
                V = vps.tile([128, L], f32, name=f"V{h}_{b}_{c}", tag=f"V{h}")
                nc.tensor.matmul(V[:], lhsT=W2[0][h], rhs=U[0],
                                 start=True, stop=False)
                nc.tensor.matmul(V[:], lhsT=W2[1][h], rhs=U[1],
                                 start=False, stop=True)
                Z = zpool.tile([128, L + 1], bf, name=f"Z{h}_{b}_{c}",
                               tag=f"Z{h}")
                if c == 0:
                    carry = z0t[:, 2 * b + h : 2 * b + h + 1]
                else:
                    carry = prev_z[b][h][:, L : L + 1]
                nc.vector.tensor_tensor_scan(
                    Z[:, 1 : L + 1], lam_bc[:, h * L : (h + 1) * L], V[:],
                    carry, mult, add,
                )
                nc.gpsimd.tensor_copy(Z[:, 0:1], carry)
                zext[h] = Z
            prev_z[b] = zext
            return U, zext

        def emit_y(c, b, U, zext, last=False):
            for g in range(2):
                Y = yps.tile([128, L], f32, name=f"Y{g}_{b}_{c}", tag=f"Y{g}")
                if g == 0:
                    nc.tensor.matmul(Y[:], lhsT=WU00, rhs=U[0],
                                     start=True, stop=False)
                else:
                    nc.tensor.matmul(Y[:], lhsT=WU01, rhs=U[0],
                                     start=True, stop=False)
                    nc.tensor.matmul(Y[:], lhsT=WU11, rhs=U[1],
                                     start=False, stop=False)
                nc.tensor.matmul(Y[:], lhsT=WC[0][g], rhs=zext[0][:, 0:L],
                                 start=False, stop=False)
                nc.tensor.matmul(Y[:], lhsT=WC[1][g], rhs=zext[1][:, 0:L],
                                 start=False, stop=True)
                Ysb = yout.tile([128, L], bf, name=f"Ysb{g}_{b}_{c}",
                                tag=f"Ysb{g}")
                ceng = nc.scalar if g == 0 else nc.vector
                oeng = nc.sync if g == 0 else nc.scalar
                dst = yT8s[b, c, 128 * g : 128 * (g + 1), :]
                if not last:
                    (nc.scalar.copy if g == 0 else nc.vector.tensor_copy)(
                        Ysb[:], Y[:])
                    oeng.dma_start(dst, Ysb[:])
                else:
                    # split the tail copy+store so the drain waits on less
                    H = L // 2
                    for p in range(2):
                        s = slice(p * H, (p + 1) * H)
                        (nc.scalar.copy if g == 0 else nc.vector.tensor_copy)(
                            Ysb[:, s], Y[:, s])
                        oeng.dma_start(dst[:, s], Ysb[:, s])

        units = [(c, b) for c in range(NCHUNK) for b in range(BPC)]
        pending = []
        for (c, b) in units:
            U, zext = emit_vscan(c, b)
            pending.append((c, b, U, zext))
            if len(pending) > 2:
                emit_y(*pending.pop(0))
        for i, p in enumerate(pending):
            emit_y(*p, last=(i == len(pending) - 1))

    _split_multi_waits(nc, mybir)
    return nc


def _host_prep(x0, u, Q, lam, Bmat, C, D):
    import ml_dtypes

    f = np.float32
    bfd = ml_dtypes.bfloat16
    lam = lam.astype(f)
    Bz = (Q.T.astype(f) @ Bmat.astype(f)).astype(f)      # (NX, NU)
    Cz = (C.astype(f) @ Q.astype(f)).astype(f)           # (NY, NX)
    z0 = (x0.astype(f) @ Q.astype(f)).astype(f)          # (B, NX)

    lam_p = np.stack([lam**j for j in range(MB)])         # (MB, NX)

    # W2[(i*32+u), n] = lam_n^(MB-1-i) * Bz[n, u]
    W2 = np.einsum("in,nu->iun", lam_p[::-1], Bz).reshape(MB * NU, NX)
    # WC[n, (32j+y)] = lam_n^j * Cz[y, n]
    WC = np.einsum("jn,yn->njy", lam_p, Cz).reshape(NX, MB * NY)
    # WU[(i*32+u), (32j+y)]
    WU = np.zeros((MB * NU, MB * NY), dtype=f)
    for j in range(MB):
        for i in range(MB):
            if i < j:
                Mji = (Cz * lam_p[j - 1 - i][None, :]) @ Bz   # (NY, NU)
                WU[i * NU : (i + 1) * NU, j * NY : (j + 1) * NY] = Mji.T
            elif i == j:
                WU[i * NU : (i + 1) * NU, j * NY : (j + 1) * NY] = D.T.astype(f)

    blocks = []
    for h in range(2):          # W2 order [g0h0, g1h0, g0h1, g1h1]
        for g in range(2):
            blocks.append(W2[128 * g : 128 * (g + 1), 128 * h : 128 * (h + 1)])
    for h in range(2):          # WC[h][g]
        for g in range(2):
            blocks.append(WC[128 * h : 128 * (h + 1), 128 * g : 128 * (g + 1)])
    # WU[g2][g] blocks; WU[1][0] is identically zero (i > j) and skipped
    blocks.append(WU[0:128, 0:128])      # WU00
    blocks.append(WU[0:128, 128:256])    # WU01
    blocks.append(WU[128:256, 128:256])  # WU11
    wAll = np.concatenate(blocks, axis=1).astype(bfd)     # (128, 11*128)

    # uT8s[b, ch, (i*32+u), k] = u[b, 8*(ch*L+k)+i, u]  (column-half major)
    uT8 = u.reshape(B, KCOL, MB, NU).transpose(0, 2, 3, 1).reshape(B, MB * NU, KCOL)
    uT8s = np.ascontiguousarray(
        uT8.reshape(B, MB * NU, 2, L).transpose(0, 2, 1, 3)
    ).astype(bfd)

    lam8 = lam**MB
    lam8c = np.stack([lam8[:128], lam8[128:]], axis=1).astype(f)  # (128, 2)
    return wAll, z0, uT8s, lam8c


def make_in_maps(x0, u, Q, lam, Bmat, C, D):
    wAll, z0, uT8s, lam8c = _host_prep(x0, u, Q, lam, Bmat, C, D)
    in_maps = []
    for cidx in range(NCORES):
        sl = slice(cidx * BPC, (cidx + 1) * BPC)
        z0_c = z0[sl]
        z0c = z0_c.reshape(BPC, 2, 128).transpose(2, 0, 1).reshape(128, 2 * BPC)
        pz = np.ascontiguousarray(np.concatenate([lam8c, z0c], axis=1))
        in_maps.append(
            {
                "uT8s": np.ascontiguousarray(uT8s[sl]),
                "wAll": wAll,
                "pz": pz,
            }
        )
    return in_maps


def kernel(x0, u, Q, lam, Bmat, C, D):
    global _PROG
    from concourse.bass_utils import run_bass_kernel_spmd

    if _PROG is None:
        _PROG = build_program()
    in_maps = make_in_maps(x0, u, Q, lam, Bmat, C, D)
    res = run_bass_kernel_spmd(_PROG, in_maps, list(range(NCORES)))
    y = np.empty((B, T, NY), dtype=np.float32)
    for cidx in range(NCORES):
        yT8s_c = res.results[cidx]["yT8s"].astype(np.float32)  # (BPC, 2, 256, L)
        # y[b, 8*(ch*L+k)+j, yy] = yT8s[b, ch, 32j+yy, k]
        y[cidx * BPC : (cidx + 1) * BPC] = (
            yT8s_c.reshape(BPC, 2, MB, NY, L)
            .transpose(0, 1, 4, 2, 3)
            .reshape(BPC, T, NY)
        )
    return y



# revision 13
# speedup vs baseline: 1.2962x; 1.0368x over previous
"""Diagonalizable linear plant (modal state-space scan) on 8 Trainium2 cores.

y[b,t] = Cz @ z[b,t-1] + D @ u[b,t],  z[b,t] = lam * z[b,t-1] + Bz @ u[b,t]
with z[b,-1] = z0[b] = x0[b] @ Q, Bz = Q^T Bmat, Cz = C Q.

Sharding: data-parallel over batch (16 batches -> 2 per core).

Block-8 formulation (the DVE scan instruction runs at ~2 cycles/element,
so the time axis is decimated 8x before it reaches the scan; everything
else is full 128x128xN=512 bf16 matmuls, fp32 PSUM):
  host packs u as uT8[(i*32+u), k] = u[8k+i, u]        (256 rows = 2 K-groups)
  PE   V_h = W2^T @ U          W2[(i,u),n] = lam_n^(7-i) Bz[n,u]
  DVE  zB = scan(lam^8, V)     block-boundary states z_{8k+7}
  PE   Y_g = WC^T @ zBprev + WU^T @ U     (g indexes (j,y) output groups)
       WC[n,(j,y)] = lam_n^j Cz[y,n]
       WU[(i,u),(j,y)] = (Cz lam^(j-1-i) Bz)[y,u] for i<j, D[y,u] for i=j, else 0
  host unpacks yT8[(32j+y), k] -> y[8k+j, y]
"""

import numpy as np

B, T, NX, NU, NY = 16, 8192, 256, 32, 32
NCORES = 8
BPC = B // NCORES   # batches per core
MB = 8              # time-block folded into matmul K
KCOL = T // MB      # block columns per batch (1024)
L = 512             # block-columns per chunk
NCHUNK = KCOL // L  # chunks per batch (2)

_PROG = None  # built Bass program, cached across kernel() calls


def _patch_tile_drain():
    """walrus codegen in this container rejects >1 sync wait on one SP
    TPB_CTRL instruction (terminal TileContext drain / NoOp). Split the
    drain's waits across preceding SP nops carrying one wait each."""
    import concourse.tile as tile
    import concourse.mybir as mybir
    from concourse.vector_clock import ScopedClock

    if getattr(tile.TileContext, "_drain_patched", False):
        return

    def _drain_and_barrier(self, tick_clock, wait_clock):
        nc = self.nc
        scratch = nc.sync.nop()
        wait_clock.add_sem_waits(
            scratch.ins, ScopedClock({None: tick_clock.global_clock})
        )
        si = scratch.ins.sync_info
        waits = list(si.on_wait) if si is not None else []
        scratch.ins.sync_info = mybir.SyncInfo(on_wait=waits[:1], on_update=[])
        for w in waits[1:]:
            n2 = nc.sync.nop()
            n2.ins.sync_info = mybir.SyncInfo(on_wait=[w], on_update=[])
        nc.sync.drain()
        nc.all_engine_barrier()
        assert self.sems is not None
        popped = nc._tile_sem_poison_stack.pop()
        assert popped is self._sem_poison
        nc.clear_and_free_semaphores(list(self.sems.allocated().values()))
        nc.all_engine_barrier()

    tile.TileContext._drain_and_barrier = _drain_and_barrier
    tile.TileContext._drain_patched = True


def _split_multi_waits(nc, mybir):
    """This container's walrus codegen accepts at most ONE sync wait per
    instruction. Hoist extra waits into standalone EventSemaphore nops on
    the same engine, placed immediately before the instruction."""
    ctr = [0]

    def fresh(engine, wait):
        ctr[0] += 1
        ev = mybir.InstEventSemaphore(name=f"I-wsplit-{ctr[0]}", ins=[], outs=[])
        ev.engine = engine
        ev.sync_info = mybir.SyncInfo(on_wait=[wait], on_update=[])
        nc.register_instruction(ev)
        return ev

    for fn in nc.m.functions:
        for bb in fn.blocks:
            out = []
            changed = False
            for inst in bb.instructions:
                si = inst.sync_info
                waits = list(si.on_wait) if si is not None else []
                if len(waits) > 1:
                    changed = True
                    for w in waits[:-1]:
                        out.append(fresh(inst.engine, w))
                    inst.sync_info = mybir.SyncInfo(
                        on_wait=[waits[-1]], on_update=list(si.on_update)
                    )
                out.append(inst)
            if changed:
                bb.instructions = out


def build_program():
    import concourse.bass as bass
    import concourse.tile as tile
    import concourse.mybir as mybir
    from contextlib import ExitStack

    _patch_tile_drain()
    f32 = mybir.dt.float32
    bf = mybir.dt.bfloat16

    nc = bass.Bass()
    # uT8s[b, ch, (i*32+u), k] = u-block columns, ch = column-half so each
    # (b, g, ch) DMA reads a fully contiguous 128 KB region (4 KB packets)
    uT8s = nc.declare_dram_parameter("uT8s", [BPC, 2, 256, L], bf, isOutput=False)
    wAll = nc.declare_dram_parameter("wAll", [128, 11 * 128], bf, isOutput=False)
    # pz: col 0:2 = lam^8 halves, cols 2: = z0 modal states (merged tiny DMA)
    pz = nc.declare_dram_parameter("pz", [128, 2 + 2 * BPC], f32, isOutput=False)
    yT8s = nc.declare_dram_parameter("yT8s", [BPC, 2, 256, L], bf, isOutput=True)

    with ExitStack() as ctx:
        tc = ctx.enter_context(tile.TileContext(nc))
        const = ctx.enter_context(tc.tile_pool(name="const", bufs=1))
        vps = ctx.enter_context(tc.tile_pool(name="vps", bufs=2, space="PSUM"))
        yps = ctx.enter_context(tc.tile_pool(name="yps", bufs=2, space="PSUM"))
        zpool = ctx.enter_context(tc.tile_pool(name="z", bufs=6))
        yout = ctx.enter_context(tc.tile_pool(name="yo", bufs=4))

        # DMA plan: the two HWDGE queues (scalar=Activation, sync=SP) carry
        # all bulk traffic, balanced so the first unit's operands (W2 for
        # h0+h1, U halves for b0) land earliest on both queues. The slow
        # software gpsimd queue gets only the Y-phase weights, which
        # stream in the background and aren't needed until ~+6us.
        W2t = const.tile([128, 512], bf)
        pzt = const.tile([128, 2 + 2 * BPC], f32)
        nc.sync.dma_start(pzt[:], pz[:])
        # weights ride the gpsimd software queue in priority order (W2 →
        # WC → WU), leaving both HWDGE queues free for pure U streaming
        nc.gpsimd.dma_start(W2t[:], wAll[:, 0:512])
        WCt = const.tile([128, 512], bf)
        nc.gpsimd.dma_start(WCt[:], wAll[:, 512:1024])
        WUt = const.tile([128, 384], bf)
        nc.gpsimd.dma_start(WUt[:], wAll[:, 1024:1408])
        lam8t = pzt[:, 0:2]
        z0t = pzt[:, 2 : 2 + 2 * BPC]
        # U tiles: [b][g] -> [128, KCOL]; column-half DMAs issued in unit
        # consumption order (b0ch0, b1ch0, b0ch1, b1ch1)
        Ubig = [[const.tile([128, KCOL], bf, name=f"U{g}b{b}") for g in range(2)]
                for b in range(BPC)]
        for ch in range(2):
            for b in range(BPC):
                sl = slice(ch * L, (ch + 1) * L)
                nc.scalar.dma_start(Ubig[b][0][:, sl], uT8s[b, ch, 0:128, :])
                nc.sync.dma_start(Ubig[b][1][:, sl], uT8s[b, ch, 128:256, :])

        # PE warm-up matmuls fill the whole DMA window back-to-back so the
        # clock governor sees sustained activity before real work begins
        dummy = const.tile([128, L], bf)
        nc.vector.memset(dummy[:], 0.0)
        WP = vps.tile([128, L], f32, name="WP", tag="V0")
        for _ in range(4):
            nc.tensor.matmul(WP[:], lhsT=dummy[:, 0:128], rhs=dummy[:],
                             start=True, stop=True)

        # lam broadcast built on DVE during the DMA fill window
        ones = const.tile([128, L], f32)
        nc.vector.memset(ones[:], 1.0)
        lam_bc = const.tile([128, 2 * L], f32)
        for h in range(2):
            nc.vector.tensor_scalar_mul(
                lam_bc[:, h * L : (h + 1) * L], ones[:], lam8t[:, h : h + 1]
            )

        def w2blk(i):
            return W2t[:, 128 * i : 128 * (i + 1)]

        def wcblk(i):
            return WCt[:, 128 * i : 128 * (i + 1)]

        # wAll W2 block order: [g0h0, g1h0, g0h1, g1h1] (h=0 pair first)
        W2 = [[w2blk(0), w2blk(2)], [w2blk(1), w2blk(3)]]      # [g][h]
        WC = [[wcblk(0), wcblk(1)], [wcblk(2), wcblk(3)]]      # [h][g]
        WU00 = WUt[:, 0:128]
        WU01 = WUt[:, 128:256]
        WU11 = WUt[:, 256:384]                                 # WU[1][0] == 0

        mult = mybir.AluOpType.mult
        add = mybir.AluOpType.add

        prev_z = [[None, None] for _ in range(BPC)]

        def emit_vscan(c, b):
            sl = slice(c * L, (c + 1) * L)
            U = [Ubig[b][0][:, sl], Ubig[b][1][:, sl]]
            zext = [None, None]
            for h in range(2):
                V = vps.tile([128, L], f32, name=f"V{h}_{b}_{c}", tag=f"V{h}")
                nc.tensor.matmul(V[:], lhsT=W2[0][h], rhs=U[0],
                                 start=True, stop=False)
                nc.tensor.matmul(V[:], lhsT=W2[1][h], rhs=U[1],
                                 start=False, stop=True)
                Z = zpool.tile([128, L + 1], bf, name=f"Z{h}_{b}_{c}",
                               tag=f"Z{h}")
                if c == 0:
                    carry = z0t[:, 2 * b + h : 2 * b + h + 1]
                else:
                    carry = prev_z[b][h][:, L : L + 1]
                nc.vector.tensor_tensor_scan(
                    Z[:, 1 : L + 1], lam_bc[:, h * L : (h + 1) * L], V[:],
                    carry, mult, add,
                )
                nc.gpsimd.tensor_copy(Z[:, 0:1], carry)
                zext[h] = Z
            prev_z[b] = zext
            return U, zext

        def emit_y(c, b, U, zext, last=False):
            for g in range(2):
                Y = yps.tile([128, L], f32, name=f"Y{g}_{b}_{c}", tag=f"Y{g}")
                nc.tensor.matmul(Y[:], lhsT=WC[0][g], rhs=zext[0][:, 0:L],
                                 start=True, stop=False)
                nc.tensor.matmul(Y[:], lhsT=WC[1][g], rhs=zext[1][:, 0:L],
                                 start=False, stop=False)
                if g == 0:
                    nc.tensor.matmul(Y[:], lhsT=WU00, rhs=U[0],
                                     start=False, stop=True)
                else:
                    nc.tensor.matmul(Y[:], lhsT=WU01, rhs=U[0],
                                     start=False, stop=False)
                    nc.tensor.matmul(Y[:], lhsT=WU11, rhs=U[1],
                                     start=False, stop=True)
                Ysb = yout.tile([128, L], bf, name=f"Ysb{g}_{b}_{c}",
                                tag=f"Ysb{g}")
                oeng = nc.sync if g == 0 else nc.scalar
                dst = yT8s[b, c, 128 * g : 128 * (g + 1), :]
                if not last:
                    nc.scalar.copy(Ysb[:], Y[:])
                    oeng.dma_start(dst, Ysb[:])
                else:
                    # split the tail copy+store so the drain waits on less
                    H = L // 2
                    for p in range(2):
                        s = slice(p * H, (p + 1) * H)
                        nc.scalar.copy(Ysb[:, s], Y[:, s])
                        oeng.dma_start(dst[:, s], Ysb[:, s])

        units = [(c, b) for c in range(NCHUNK) for b in range(BPC)]
        pending = []
        for (c, b) in units:
            U, zext = emit_vscan(c, b)
            pending.append((c, b, U, zext))
            if len(pending) > 2:
                emit_y(*pending.pop(0))
        for i, p in enumerate(pending):
            emit_y(*p, last=(i == len(pending) - 1))

    _split_multi_waits(nc, mybir)
    return nc


def _host_prep(x0, u, Q, lam, Bmat, C, D):
    import ml_dtypes

    f = np.float32
    bfd = ml_dtypes.bfloat16
    lam = lam.astype(f)
    Bz = (Q.T.astype(f) @ Bmat.astype(f)).astype(f)      # (NX, NU)
    Cz = (C.astype(f) @ Q.astype(f)).astype(f)           # (NY, NX)
    z0 = (x0.astype(f) @ Q.astype(f)).astype(f)          # (B, NX)

    lam_p = np.stack([lam**j for j in range(MB)])         # (MB, NX)

    # W2[(i*32+u), n] = lam_n^(MB-1-i) * Bz[n, u]
    W2 = np.einsum("in,nu->iun", lam_p[::-1], Bz).reshape(MB * NU, NX)
    # WC[n, (32j+y)] = lam_n^j * Cz[y, n]
    WC = np.einsum("jn,yn->njy", lam_p, Cz).reshape(NX, MB * NY)
    # WU[(i*32+u), (32j+y)]
    WU = np.zeros((MB * NU, MB * NY), dtype=f)
    for j in range(MB):
        for i in range(MB):
            if i < j:
                Mji = (Cz * lam_p[j - 1 - i][None, :]) @ Bz   # (NY, NU)
                WU[i * NU : (i + 1) * NU, j * NY : (j + 1) * NY] = Mji.T
            elif i == j:
                WU[i * NU : (i + 1) * NU, j * NY : (j + 1) * NY] = D.T.astype(f)

    blocks = []
    for h in range(2):          # W2 order [g0h0, g1h0, g0h1, g1h1]
        for g in range(2):
            blocks.append(W2[128 * g : 128 * (g + 1), 128 * h : 128 * (h + 1)])
    for h in range(2):          # WC[h][g]
        for g in range(2):
            blocks.append(WC[128 * h : 128 * (h + 1), 128 * g : 128 * (g + 1)])
    # WU[g2][g] blocks; WU[1][0] is identically zero (i > j) and skipped
    blocks.append(WU[0:128, 0:128])      # WU00
    blocks.append(WU[0:128, 128:256])    # WU01
    blocks.append(WU[128:256, 128:256])  # WU11
    wAll = np.concatenate(blocks, axis=1).astype(bfd)     # (128, 11*128)

    # uT8s[b, ch, (i*32+u), k] = u[b, 8*(ch*L+k)+i, u]  (column-half major)
    uT8 = u.reshape(B, KCOL, MB, NU).transpose(0, 2, 3, 1).reshape(B, MB * NU, KCOL)
    uT8s = np.ascontiguousarray(
        uT8.reshape(B, MB * NU, 2, L).transpose(0, 2, 1, 3)
    ).astype(bfd)

    lam8 = lam**MB
    lam8c = np.stack([lam8[:128], lam8[128:]], axis=1).astype(f)  # (128, 2)
    return wAll, z0, uT8s, lam8c


def make_in_maps(x0, u, Q, lam, Bmat, C, D):
    wAll, z0, uT8s, lam8c = _host_prep(x0, u, Q, lam, Bmat, C, D)
    in_maps = []
    for cidx in range(NCORES):
        sl = slice(cidx * BPC, (cidx + 1) * BPC)
        z0_c = z0[sl]
        z0c = z0_c.reshape(BPC, 2, 128).transpose(2, 0, 1).reshape(128, 2 * BPC)
        pz = np.ascontiguousarray(np.concatenate([lam8c, z0c], axis=1))
        in_maps.append(
            {
                "uT8s": np.ascontiguousarray(uT8s[sl]),
                "wAll": wAll,
                "pz": pz,
            }
        )
    return in_maps


def kernel(x0, u, Q, lam, Bmat, C, D):
    global _PROG
    from concourse.bass_utils import run_bass_kernel_spmd

    if _PROG is None:
        _PROG = build_program()
    in_maps = make_in_maps(x0, u, Q, lam, Bmat, C, D)
    res = run_bass_kernel_spmd(_PROG, in_maps, list(range(NCORES)))
    y = np.empty((B, T, NY), dtype=np.float32)
    for cidx in range(NCORES):
        yT8s_c = res.results[cidx]["yT8s"].astype(np.float32)  # (BPC, 2, 256, L)
        # y[b, 8*(ch*L+k)+j, yy] = yT8s[b, ch, 32j+yy, k]
        y[cidx * BPC : (cidx + 1) * BPC] = (
            yT8s_c.reshape(BPC, 2, MB, NY, L)
            .transpose(0, 1, 4, 2, 3)
            .reshape(BPC, T, NY)
        )
    return y

